# revision 31
# baseline (speedup 1.0000x reference)
"""Trainium2 Bass kernel: per-aspect windowed attention (sparse_attention).

Reference math:
    proj[a,b,s,f] = sum_h doc[b,s,h] aspProj[a,h,f]
    score[a,b,s]  = sum_{w,f} proj[a,b,s+w-2,f] E[a,f,w]      (zero-padded in s)
    attn          = softmax_s(score)
    rep[a,b,f]    = sum_s proj[a,b,s,f] attn[a,b,s]

Algebraic refactor (proj never materialized):
    K[a,h,w]     = sum_f aspProj[a,h,f] E[a,f,w]              (tiny, host-side)
    score[a,b,s] = sum_{w,h} doc[b,s+w-2,h] K[a,h,w]
    attn         = exp(score) / rowsum                         (scores are tiny)
    rep[a,b,f]   = sum_h (sum_s attn[a,b,s] doc[b,s,h]) aspProj[a,h,f]

Sharding: data-parallel over batch, 8 batches per NeuronCore x 8 cores, no
collectives. Host pre-packs doc in bf16 twice: natural [s,h] tiles for the
s-contraction (wdoc) and transposed+padded [h,s] for the h-contraction
(scores). Raw bacc (no Tile framework) with hand-placed semaphores.

Score matmuls are 4-way column-tiled: batches j=0..3 of a quad run
concurrently in PE column groups (tile_position=(0,32j)), writing score
rows at partition 32j of one [128, S] psum pair. That packs M=8 output
rows x4 into the array and lets the softmax (exp+rowsum, reciprocal,
scale) run 128 partitions wide -- one op per quad instead of eight narrow
ones. Unused psum rows are zero-filled by the warmup matmuls so the
batched exp reads clean data.

attn is produced in bf16 (attn_out dram is bf16, upcast on host); scores
are tiny so exp(score) ~ 1 +- 0.2 and bf16 keeps ~3 decimal digits, well
inside the accuracy budget.
"""

import numpy as np
import ml_dtypes

import concourse.bass as bass
import concourse.bacc as bacc
import concourse.mybir as mybir
from concourse.bass_utils import run_bass_kernel_spmd

B, S, H, A, W = 64, 1024, 128, 8, 5
PAD = (W - 1) // 2
NCORES = 8
BL = B // NCORES          # local batches per core
SP = S + 2 * PAD          # padded seq width of docT
NPAIR = BL * A            # 64 (batch, aspect) pairs per core
ST = S // 128             # seq tiles per batch

BF16 = mybir.dt.bfloat16
FP32 = mybir.dt.float32

TRACE = False             # test.py flips this to profile
LAST_RESULT = None
N_WARMUP = 8              # N=512 warmup matmuls (~3.4us cold = HAM window)

_NC_CACHE = None


def _build_nc():
    nc = bacc.Bacc(
        "TRN2", target_bir_lowering=False, debug=False, num_devices=NCORES
    )
    Exp = mybir.ActivationFunctionType.Exp

    kwid_d = nc.dram_tensor("kwid", [128, 200], BF16, kind="ExternalInput")
    apw_d = nc.dram_tensor("apw", [128, 1024], BF16, kind="ExternalInput")
    dT_d = [
        nc.dram_tensor("docTA", [4, 128, SP], BF16, kind="ExternalInput"),
        nc.dram_tensor("docTB", [4, 128, SP], BF16, kind="ExternalInput"),
    ]
    dN_d = [
        nc.dram_tensor("docNA", [128, 4, ST, H], BF16, kind="ExternalInput"),
        nc.dram_tensor("docNB", [128, 4, ST, H], BF16, kind="ExternalInput"),
    ]
    attn_o = nc.dram_tensor("attn_out", [NPAIR, S], BF16, kind="ExternalOutput")
    rep_o = nc.dram_tensor("rep_out", [BL, A, H], FP32, kind="ExternalOutput")

    # ---- SBUF ----
    wt = nc.alloc_sbuf_tensor("wt_sb", [128, 1224], BF16)
    dT = [nc.alloc_sbuf_tensor(f"dT{i}", [128, 4, SP], BF16) for i in range(2)]
    dN = [
        nc.alloc_sbuf_tensor(f"dN{i}", [128, 4, ST, H], BF16) for i in range(2)
    ]
    probs = [
        nc.alloc_sbuf_tensor(f"probs{q}", [128, S], FP32) for q in range(2)
    ]
    attn_q = [
        nc.alloc_sbuf_tensor(f"attnq{q}", [128, S], BF16) for q in range(2)
    ]
    attn_ball = nc.alloc_sbuf_tensor("attn_ball", [NPAIR, S], BF16)
    attnT = nc.alloc_sbuf_tensor("attnT", [128, ST, NPAIR], BF16)
    sums_big = nc.alloc_sbuf_tensor("sums_big", [128, 2], FP32)
    rs_big = nc.alloc_sbuf_tensor("rs_big", [128, 2], FP32)
    zb = nc.alloc_sbuf_tensor("zb", [128, 1], FP32)
    scr = nc.alloc_sbuf_tensor("scr", [128, 1], FP32)
    wtmp_q = [
        nc.alloc_sbuf_tensor(f"wtmp{q}", [128, H], BF16) for q in range(2)
    ]
    wtmp_all = nc.alloc_sbuf_tensor("wtmp_all", [A, BL, H], BF16)
    junk = nc.alloc_sbuf_tensor("junk", [128, 512], BF16)
    wallT = nc.alloc_sbuf_tensor("wallT", [H, NPAIR], BF16)
    rep_all = nc.alloc_sbuf_tensor("rep_all", [BL, A, H], FP32)

    idv = wt.ap()[:, 0:128]
    kwv = wt.ap()[:, 128 : 128 + W * A].rearrange("h (w a) -> h w a", a=A)
    selv = wt.ap()[:, 168:200]
    apv = wt.ap()[:, 200 : 200 + A * H].rearrange("h (a f) -> h a f", f=H)
    wallT_v = wallT.ap().rearrange("h (b a) -> h b a", a=A)

    # ---- PSUM: quadA scores banks 0-1, quadB scores banks 2-3, warmup
    # bank 4 (also zero-fills the score banks), wdoc/rep rotation 5-7.
    # psT0/psT1/psWT alias the score banks (dead once exp has run).
    ps_q = [
        nc.place_psum_tensor(f"ps_q{q}", [128, S], FP32, bank=2 * q)
        for q in range(2)
    ]
    ps_warm = nc.place_psum_tensor("ps_warm", [128, 512], FP32, bank=4)
    ps_wq = [
        nc.place_psum_tensor(f"ps_wq{q}", [128, H], FP32, bank=5 + q)
        for q in range(2)
    ]
    ps_rep = [
        nc.place_psum_tensor("ps_rep0", [A, H], FP32, bank=7),
        nc.place_psum_tensor("ps_rep1", [A, H], FP32, bank=4),
    ]
    ps_cpt = [
        nc.place_psum_tensor(f"ps_cpt{q}", [32, S], FP32, bank=2 * q)
        for q in range(2)
    ]
    psT = nc.place_psum_tensor("psT", [128, ST, 32], BF16, bank=4)
    psWT = nc.place_psum_tensor("psWT", [128, NPAIR], BF16, bank=2)

    sems = {}
    sem_names = (
        ["kwid", "apw", "dNA", "dNB",
         "z", "j", "mm", "exp", "rcp", "mul", "cpt", "bcp", "tp", "ttc",
         "wd", "wcp", "wtp", "wtc", "rep", "repcv", "repcs", "oattn", "orep"]
        + [f"dTA{j}" for j in range(4)]
        + [f"dTB{j}" for j in range(4)]
    )
    for name in sem_names:
        sems[name] = nc.alloc_semaphore(f"S_{name}")

    with nc.Block() as block:

        @block.sync
        def _(sync):
            sync.dma_start(wt.ap()[:, 0:200], kwid_d.ap()[:]).then_inc(
                sems["kwid"], 16
            )
            for i, nm in ((0, "dTA"), (1, "dTB")):
                for j in range(4):
                    sync.dma_start(
                        dT[i].ap()[:, j, :], dT_d[i].ap()[j]
                    ).then_inc(sems[f"{nm}{j}"], 16)
            sync.dma_start(dN[0].ap()[:], dN_d[0].ap()[:]).then_inc(
                sems["dNA"], 16
            )
            sync.dma_start(dN[1].ap()[:], dN_d[1].ap()[:]).then_inc(
                sems["dNB"], 16
            )
            sync.dma_start(wt.ap()[:, 200:1224], apw_d.ap()[:]).then_inc(
                sems["apw"], 16
            )
            # single attn output DMA (bf16; host upcasts)
            sync.wait_ge(sems["bcp"], 2)
            sync.dma_start(attn_o.ap()[:], attn_ball.ap()[:]).then_inc(
                sems["oattn"], 16
            )
            sync.wait_ge(sems["repcv"], 4)
            sync.wait_ge(sems["repcs"], 4)
            sync.dma_start(rep_o.ap()[:], rep_all.ap()[:]).then_inc(
                sems["orep"], 16
            )
            sync.wait_ge(sems["oattn"], 16)
            sync.wait_ge(sems["orep"], 16)

        @block.tensor
        def _(pe):
            te = nc.tensor
            # warmup on a zeroed junk tile: un-throttles the HAM while the
            # input DMAs fly, and zero-fills the psum banks that the
            # 128-wide consumers later read
            pe.wait_ge(sems["j"], 1)
            warm_tgts = [
                ps_q[0].ap()[:, 0:512],
                ps_q[0].ap()[:, 512:1024],
                ps_q[1].ap()[:, 0:512],
                ps_q[1].ap()[:, 512:1024],
                ps_wq[0].ap()[:, :],
                ps_wq[1].ap()[:, :],
            ]
            for i in range(N_WARMUP):
                tgt = warm_tgts[i] if i < 6 else ps_warm.ap()[:, :]
                te.matmul(
                    tgt,
                    junk.ap()[:, 0:128],
                    junk.ap()[:, 0 : tgt.shape[-1]],
                )
            # ---- scores: per quad, 4 batches run in parallel PE column
            # groups; 10 accumulating MMs per batch
            pe.wait_ge(sems["kwid"], 16)
            for q in range(2):
                for j in range(4):
                    pe.wait_ge(sems[("dTA" if q == 0 else "dTB") + str(j)], 16)
                for w in range(W):
                    for half in range(2):
                        for j in range(4):
                            mm = te.matmul(
                                ps_q[q].ap()[
                                    32 * j : 32 * j + A,
                                    half * 512 : half * 512 + 512,
                                ],
                                kwv[:, w, :],
                                dT[q].ap()[
                                    :, j, half * 512 + w : half * 512 + w + 512
                                ],
                                start=(w == 0),
                                stop=(w == W - 1),
                                tile_position=(0, 32 * j),
                                skip_group_check=True,
                            )
                mm.then_inc(sems["mm"], 1)
            # ---- compact quad A rows 32j+a -> contiguous pairs (psum)
            pe.wait_ge(sems["mul"], 1)
            for half in range(2):
                te.matmul(
                    ps_cpt[0].ap()[:, half * 512 : half * 512 + 512],
                    selv,
                    attn_q[0].ap()[:, half * 512 : half * 512 + 512],
                ).then_inc(sems["cpt"], 1)
            # ---- lo-half attn transposes
            pe.wait_ge(sems["bcp"], 1)
            for t in range(ST):
                te.matmul(
                    psT.ap()[:, t, :],
                    attn_ball.ap()[0:32, t * 128 : (t + 1) * 128],
                    idv[0:32, 0:32],
                    is_transpose=True,
                ).then_inc(sems["tp"], 1)
            # ---- compact quad B (ready during wdoc-lo)
            pe.wait_ge(sems["mul"], 2)
            for half in range(2):
                te.matmul(
                    ps_cpt[1].ap()[:, half * 512 : half * 512 + 512],
                    selv,
                    attn_q[1].ap()[:, half * 512 : half * 512 + 512],
                ).then_inc(sems["cpt"], 1)
            # ---- wdoc lo (b=0..3), 4-way column-tiled
            pe.wait_ge(sems["dNA"], 16)
            pe.wait_ge(sems["ttc"], 1)
            for t in range(ST):
                for j in range(4):
                    mm = te.matmul(
                        ps_wq[0].ap()[32 * j : 32 * j + A, :],
                        attnT.ap()[:, t, j * A : (j + 1) * A],
                        dN[0].ap()[:, j, t, :],
                        start=(t == 0),
                        stop=(t == ST - 1),
                        tile_position=(0, 32 * j),
                        skip_group_check=True,
                    )
            mm.then_inc(sems["wd"], 1)
            # ---- hi-half attn transposes
            pe.wait_ge(sems["bcp"], 2)
            for t in range(ST):
                te.matmul(
                    psT.ap()[:, t, :],
                    attn_ball.ap()[32:64, t * 128 : (t + 1) * 128],
                    idv[32:64, 32:64],
                    is_transpose=True,
                ).then_inc(sems["tp"], 1)
            # ---- wdoc hi
            pe.wait_ge(sems["dNB"], 16)
            pe.wait_ge(sems["ttc"], 2)
            for t in range(ST):
                for j in range(4):
                    mm = te.matmul(
                        ps_wq[1].ap()[32 * j : 32 * j + A, :],
                        attnT.ap()[:, t, (4 + j) * A : (5 + j) * A],
                        dN[1].ap()[:, j, t, :],
                        start=(t == 0),
                        stop=(t == ST - 1),
                        tile_position=(0, 32 * j),
                        skip_group_check=True,
                    )
            mm.then_inc(sems["wd"], 1)
            # ---- wall transposes, straight from the cast buffers
            for b in range(BL):
                pe.wait_ge(sems["wcp"], b + 1)
                te.matmul(
                    psWT.ap()[:, b * A : (b + 1) * A],
                    wtmp_all.ap()[:, b, :],
                    idv[0:A, 0:A],
                    is_transpose=True,
                ).then_inc(sems["wtp"], 1)
            # ---- rep
            pe.wait_ge(sems["wtc"], 1)
            pe.wait_ge(sems["apw"], 16)
            for a in range(A):
                if a >= 2:
                    prev = a - 2
                    if prev % 2 == 0:
                        pe.wait_ge(sems["repcv"], prev // 2 + 1)
                    else:
                        pe.wait_ge(sems["repcs"], prev // 2 + 1)
                te.matmul(
                    ps_rep[a % 2].ap()[:, :], wallT_v[:, :, a], apv[:, a, :]
                ).then_inc(sems["rep"], 1)

        @block.scalar
        def _(act):
            sc = nc.scalar
            act.wait_ge(sems["z"], 1)
            # dummy exp so the ACT table load happens before the scores land
            sc.activation(scr.ap()[:], zb.ap()[:], Exp, bias=zb.ap()[:, :])
            for q in range(2):
                act.wait_ge(sems["mm"], q + 1)
                sc.activation(
                    probs[q].ap()[:],
                    ps_q[q].ap()[:, :],
                    Exp,
                    bias=zb.ap()[:, :],
                    accum_out=sums_big.ap()[:, q : q + 1],
                ).then_inc(sems["exp"], 1)
                # 128-wide softmax scale (ACT is ~2.5x faster than DVE here)
                act.wait_ge(sems["rcp"], q + 1)
                sc.activation(
                    attn_q[q].ap()[:],
                    probs[q].ap()[:],
                    mybir.ActivationFunctionType.Copy,
                    scale=rs_big.ap()[:, q : q + 1],
                ).then_inc(sems["mul"], 1)
            # rep psum -> sbuf copies, odd aspects (even on DVE)
            for a in range(1, A, 2):
                act.wait_ge(sems["rep"], a + 1)
                sc.copy(rep_all.ap()[:, a, :], ps_rep[a % 2].ap()[:, :]).then_inc(
                    sems["repcs"], 1
                )

        @block.vector
        def _(dve):
            v = nc.vector
            v.memset(junk.ap()[:], 0.0).then_inc(sems["j"], 1)
            for q in range(2):
                dve.wait_ge(sems["exp"], q + 1)
                v.reciprocal(
                    rs_big.ap()[:, q : q + 1], sums_big.ap()[:, q : q + 1]
                ).then_inc(sems["rcp"], 1)
            # compacted rows -> attn_ball (legal bases 0/32)
            dve.wait_ge(sems["cpt"], 2)
            v.tensor_copy(attn_ball.ap()[0:32, :], ps_cpt[0].ap()[:, :]).then_inc(
                sems["bcp"], 1
            )
            # lo-half: copy the PE transposes out of PSUM
            dve.wait_ge(sems["tp"], ST)
            v.tensor_copy(attnT.ap()[:, :, 0:32], psT.ap()[:, :, :]).then_inc(
                sems["ttc"], 1
            )
            dve.wait_ge(sems["wd"], 1)
            for j in range(4):
                v.tensor_copy(
                    wtmp_all.ap()[:, j, :], ps_wq[0].ap()[32 * j : 32 * j + A, :]
                ).then_inc(sems["wcp"], 1)
            dve.wait_ge(sems["cpt"], 4)
            v.tensor_copy(attn_ball.ap()[32:64, :], ps_cpt[1].ap()[:, :]).then_inc(
                sems["bcp"], 1
            )
            dve.wait_ge(sems["tp"], 2 * ST)
            v.tensor_copy(attnT.ap()[:, :, 32:64], psT.ap()[:, :, :]).then_inc(
                sems["ttc"], 1
            )
            dve.wait_ge(sems["wd"], 2)
            for j in range(4):
                v.tensor_copy(
                    wtmp_all.ap()[:, 4 + j, :], ps_wq[1].ap()[32 * j : 32 * j + A, :]
                ).then_inc(sems["wcp"], 1)
            # wallT out of PSUM
            dve.wait_ge(sems["wtp"], BL)
            v.tensor_copy(wallT.ap()[:], psWT.ap()[:]).then_inc(sems["wtc"], 1)
            # rep psum -> sbuf copies, even aspects
            for a in range(0, A, 2):
                dve.wait_ge(sems["rep"], a + 1)
                v.tensor_copy(
                    rep_all.ap()[:, a, :], ps_rep[a % 2].ap()[:, :]
                ).then_inc(sems["repcv"], 1)

        @block.gpsimd
        def _(gp):
            nc.gpsimd.memset(zb.ap()[:], 0.0).then_inc(sems["z"], 1)

    nc.compile()
    return nc


def _get_nc():
    global _NC_CACHE
    if _NC_CACHE is None:
        _NC_CACHE = _build_nc()
    return _NC_CACHE


def make_in_maps(doc, aspE, aP):
    E = aspE.reshape(A, H, W)
    K = np.einsum("ahx,axw->ahw", aP, E)  # (A, H, W)
    kwid = np.zeros((128, 200), dtype=ml_dtypes.bfloat16)
    kwid[:, 0:128] = np.eye(128)
    kwid[:, 128 : 128 + W * A] = (
        K.transpose(1, 2, 0).reshape(H, W * A).astype(ml_dtypes.bfloat16)
    )
    for j in range(4):
        for a in range(A):
            kwid[32 * j + a, 168 + 8 * j + a] = 1
    apw = np.ascontiguousarray(aP.transpose(1, 0, 2).reshape(H, A * H)).astype(
        ml_dtypes.bfloat16
    )

    doc_bf = doc.astype(ml_dtypes.bfloat16)  # (B, S, H)
    in_maps = []
    for c in range(NCORES):
        dc = doc_bf[c * BL : (c + 1) * BL]  # (BL, S, H)
        m = {"kwid": kwid, "apw": apw}
        for i, half in enumerate(("A", "B")):
            dh = dc[i * 4 : (i + 1) * 4]
            # docN[p, b, t, h] = doc[b, t*128+p, h]
            m[f"docN{half}"] = np.ascontiguousarray(
                dh.reshape(4, ST, 128, H).transpose(2, 0, 1, 3)
            )
            dTc = np.zeros((4, 128, SP), dtype=ml_dtypes.bfloat16)
            dTc[:, :, PAD : PAD + S] = dh.transpose(0, 2, 1)
            m[f"docT{half}"] = dTc
        in_maps.append(m)
    return in_maps


def kernel(batch_docIn, aspEmbed_weight, aspProj):
    global LAST_RESULT
    doc = np.asarray(batch_docIn, dtype=np.float32)
    aspE = np.asarray(aspEmbed_weight, dtype=np.float32)
    aP = np.asarray(aspProj, dtype=np.float32)
    in_maps = make_in_maps(doc, aspE, aP)

    nc = _get_nc()
    res = run_bass_kernel_spmd(
        nc, in_maps, core_ids=list(range(NCORES)), trace=TRACE
    )
    LAST_RESULT = res

    attn = np.empty((B, A, S), dtype=np.float32)
    rep = np.empty((B, A, H), dtype=np.float32)
    for c in range(NCORES):
        attn[c * BL : (c + 1) * BL] = (
            res.results[c]["attn_out"].astype(np.float32).reshape(BL, A, S)
        )
        rep[c * BL : (c + 1) * BL] = res.results[c]["rep_out"]
    return attn, rep


# revision 33
# speedup vs baseline: 1.0852x; 1.0852x over previous
"""Trainium2 Bass kernel: per-aspect windowed attention (sparse_attention).

Reference math:
    proj[a,b,s,f] = sum_h doc[b,s,h] aspProj[a,h,f]
    score[a,b,s]  = sum_{w,f} proj[a,b,s+w-2,f] E[a,f,w]      (zero-padded in s)
    attn          = softmax_s(score)
    rep[a,b,f]    = sum_s proj[a,b,s,f] attn[a,b,s]

Algebraic refactor (proj never materialized):
    K[a,h,w]     = sum_f aspProj[a,h,f] E[a,f,w]              (tiny, host-side)
    score[a,b,s] = sum_{w,h} doc[b,s+w-2,h] K[a,h,w]
    attn         = exp(score) / rowsum                         (scores are tiny)
    rep[a,b,f]   = sum_h (sum_s attn[a,b,s] doc[b,s,h]) aspProj[a,h,f]

Sharding: data-parallel over batch, 8 batches per NeuronCore x 8 cores, no
collectives. Host pre-packs doc in bf16 twice: natural [s,h] tiles for the
s-contraction (wdoc) and transposed+padded [h,s] for the h-contraction
(scores). Raw bacc (no Tile framework) with hand-placed semaphores.

Score matmuls are 4-way column-tiled: batches j=0..3 of a quad run
concurrently in PE column groups (tile_position=(0,32j)), writing score
rows at partition 32j of one [128, S] psum pair. That packs M=8 output
rows x4 into the array and lets the softmax (exp+rowsum, reciprocal,
scale) run 128 partitions wide -- one op per quad instead of eight narrow
ones. Unused psum rows are zero-filled by the warmup matmuls so the
batched exp reads clean data.

attn is produced in bf16 (attn_out dram is bf16, upcast on host); scores
are tiny so exp(score) ~ 1 +- 0.2 and bf16 keeps ~3 decimal digits, well
inside the accuracy budget.
"""

import numpy as np
import ml_dtypes

import concourse.bass as bass
import concourse.bacc as bacc
import concourse.mybir as mybir
from concourse.bass_utils import run_bass_kernel_spmd

B, S, H, A, W = 64, 1024, 128, 8, 5
PAD = (W - 1) // 2
NCORES = 8
BL = B // NCORES          # local batches per core
SP = S + 2 * PAD          # padded seq width of docT
NPAIR = BL * A            # 64 (batch, aspect) pairs per core
ST = S // 128             # seq tiles per batch

BF16 = mybir.dt.bfloat16
FP32 = mybir.dt.float32

TRACE = False             # test.py flips this to profile
LAST_RESULT = None
N_WARMUP = 8              # N=512 warmup matmuls (~3.4us cold = HAM window)

_NC_CACHE = None


def _build_nc():
    nc = bacc.Bacc(
        "TRN2", target_bir_lowering=False, debug=False, num_devices=NCORES
    )
    Exp = mybir.ActivationFunctionType.Exp

    kwid_d = nc.dram_tensor("kwid", [128, 200], BF16, kind="ExternalInput")
    apw_d = nc.dram_tensor("apw", [128, 1024], BF16, kind="ExternalInput")
    dT_d = [
        nc.dram_tensor("docTA", [4, 128, SP], BF16, kind="ExternalInput"),
        nc.dram_tensor("docTB", [4, 128, SP], BF16, kind="ExternalInput"),
    ]
    dN_d = [
        nc.dram_tensor("docNA", [128, 4, ST, H], BF16, kind="ExternalInput"),
        nc.dram_tensor("docNB", [128, 4, ST, H], BF16, kind="ExternalInput"),
    ]
    attn_o = nc.dram_tensor("attn_out", [NPAIR, S], BF16, kind="ExternalOutput")
    rep_o = nc.dram_tensor("rep_out", [BL, A, H], FP32, kind="ExternalOutput")

    # ---- SBUF ----
    wt = nc.alloc_sbuf_tensor("wt_sb", [128, 1224], BF16)
    dT = [
        [nc.alloc_sbuf_tensor(f"dT{i}_{j}", [128, SP], BF16) for j in range(4)]
        for i in range(2)
    ]
    dN = [
        nc.alloc_sbuf_tensor(f"dN{i}", [128, 4, ST, H], BF16) for i in range(2)
    ]
    probs = [
        nc.alloc_sbuf_tensor(f"probs{q}", [128, S], FP32) for q in range(2)
    ]
    attn_q = [
        nc.alloc_sbuf_tensor(f"attnq{q}", [128, S], BF16) for q in range(2)
    ]
    attn_ball = nc.alloc_sbuf_tensor("attn_ball", [NPAIR, S], BF16)
    attnT = nc.alloc_sbuf_tensor("attnT", [128, ST, NPAIR], BF16)
    sums_big = nc.alloc_sbuf_tensor("sums_big", [128, 2], FP32)
    rs_big = nc.alloc_sbuf_tensor("rs_big", [128, 2], FP32)
    zb = nc.alloc_sbuf_tensor("zb", [128, 1], FP32)
    scr = nc.alloc_sbuf_tensor("scr", [128, 1], FP32)
    wtmp_q = [
        nc.alloc_sbuf_tensor(f"wtmp{q}", [128, H], BF16) for q in range(2)
    ]
    wtmp_all = nc.alloc_sbuf_tensor("wtmp_all", [A, BL, H], BF16)
    junk = nc.alloc_sbuf_tensor("junk", [128, 512], BF16)
    wallT = nc.alloc_sbuf_tensor("wallT", [H, NPAIR], BF16)
    rep_all = nc.alloc_sbuf_tensor("rep_all", [BL, A, H], FP32)

    idv = wt.ap()[:, 0:128]
    kwv = wt.ap()[:, 128 : 128 + W * A].rearrange("h (w a) -> h w a", a=A)
    selv = wt.ap()[:, 168:200]
    apv = wt.ap()[:, 200 : 200 + A * H].rearrange("h (a f) -> h a f", f=H)
    wallT_v = wallT.ap().rearrange("h (b a) -> h b a", a=A)

    # ---- PSUM: quadA scores banks 0-1, quadB scores banks 2-3, warmup
    # bank 4 (also zero-fills the score banks), wdoc/rep rotation 5-7.
    # psT0/psT1/psWT alias the score banks (dead once exp has run).
    ps_q = [
        nc.place_psum_tensor(f"ps_q{q}", [128, S], FP32, bank=2 * q)
        for q in range(2)
    ]
    ps_warm = nc.place_psum_tensor("ps_warm", [128, 512], FP32, bank=4)
    ps_wq = [
        nc.place_psum_tensor(f"ps_wq{q}", [128, H], FP32, bank=5 + q)
        for q in range(2)
    ]
    ps_rep = [
        nc.place_psum_tensor("ps_rep0", [A, H], FP32, bank=7),
        nc.place_psum_tensor("ps_rep1", [A, H], FP32, bank=4),
    ]
    ps_cpt = [
        nc.place_psum_tensor(f"ps_cpt{q}", [32, S], FP32, bank=2 * q)
        for q in range(2)
    ]
    psT = nc.place_psum_tensor("psT", [128, ST, 32], BF16, bank=4)
    psWT = nc.place_psum_tensor("psWT", [128, NPAIR], BF16, bank=2)

    sems = {}
    sem_names = (
        ["kwid", "apw", "dNA", "dNB",
         "z", "j", "mm", "exp", "rcp", "mul", "cpt", "bcp", "tp", "ttc",
         "wd", "wcp", "wtp", "wtc", "rep", "repcv", "repcs", "oattn", "orep"]
        + [f"dTA{j}" for j in range(4)]
        + [f"dTB{j}" for j in range(4)]
    )
    for name in sem_names:
        sems[name] = nc.alloc_semaphore(f"S_{name}")

    with nc.Block() as block:

        @block.sync
        def _(sync):
            sync.dma_start(wt.ap()[:, 0:200], kwid_d.ap()[:]).then_inc(
                sems["kwid"], 16
            )
            for i, nm in ((0, "dTA"), (1, "dTB")):
                for j in range(4):
                    sync.dma_start(
                        dT[i][j].ap()[:], dT_d[i].ap()[j]
                    ).then_inc(sems[f"{nm}{j}"], 16)
            sync.dma_start(dN[0].ap()[:], dN_d[0].ap()[:]).then_inc(
                sems["dNA"], 16
            )
            sync.dma_start(dN[1].ap()[:], dN_d[1].ap()[:]).then_inc(
                sems["dNB"], 16
            )
            sync.dma_start(wt.ap()[:, 200:1224], apw_d.ap()[:]).then_inc(
                sems["apw"], 16
            )
            # single attn output DMA (bf16; host upcasts)
            sync.wait_ge(sems["bcp"], 4)
            sync.dma_start(attn_o.ap()[:], attn_ball.ap()[:]).then_inc(
                sems["oattn"], 16
            )
            sync.wait_ge(sems["repcv"], 4)
            sync.wait_ge(sems["repcs"], 4)
            sync.dma_start(rep_o.ap()[:], rep_all.ap()[:]).then_inc(
                sems["orep"], 16
            )
            sync.wait_ge(sems["oattn"], 16)
            sync.wait_ge(sems["orep"], 16)

        @block.tensor
        def _(pe):
            te = nc.tensor
            # warmup on a zeroed junk tile: un-throttles the HAM while the
            # input DMAs fly, and zero-fills the psum banks that the
            # 128-wide consumers later read
            pe.wait_ge(sems["j"], 1)
            warm_tgts = [
                ps_q[0].ap()[:, 0:512],
                ps_q[0].ap()[:, 512:1024],
                ps_q[1].ap()[:, 0:512],
                ps_q[1].ap()[:, 512:1024],
                ps_wq[0].ap()[:, :],
                ps_wq[1].ap()[:, :],
            ]
            for i in range(N_WARMUP):
                tgt = warm_tgts[i] if i < 6 else ps_warm.ap()[:, :]
                te.matmul(
                    tgt,
                    junk.ap()[:, 0:128],
                    junk.ap()[:, 0 : tgt.shape[-1]],
                )
            # ---- scores: per quad, 4 batches run in parallel PE column
            # groups; 10 accumulating MMs per batch
            pe.wait_ge(sems["kwid"], 16)
            for q in range(2):
                for j in range(4):
                    pe.wait_ge(sems[("dTA" if q == 0 else "dTB") + str(j)], 16)
                for w in range(W):
                    for half in range(2):
                        for j in range(4):
                            mm = te.matmul(
                                ps_q[q].ap()[
                                    32 * j : 32 * j + A,
                                    half * 512 : half * 512 + 512,
                                ],
                                kwv[:, w, :],
                                dT[q][j].ap()[
                                    :, half * 512 + w : half * 512 + w + 512
                                ],
                                start=(w == 0),
                                stop=(w == W - 1),
                                tile_position=(0, 32 * j),
                                skip_group_check=True,
                            )
                mm.then_inc(sems["mm"], 1)
            # ---- compact quad A rows 32j+a -> contiguous pairs (psum)
            pe.wait_ge(sems["mul"], 1)
            for half in range(2):
                te.matmul(
                    ps_cpt[0].ap()[:, half * 512 : half * 512 + 512],
                    selv,
                    attn_q[0].ap()[:, half * 512 : half * 512 + 512],
                ).then_inc(sems["cpt"], 1)
            # ---- lo-half attn transposes
            pe.wait_ge(sems["bcp"], 2)
            for t in range(ST):
                te.matmul(
                    psT.ap()[:, t, :],
                    attn_ball.ap()[0:32, t * 128 : (t + 1) * 128],
                    idv[0:32, 0:32],
                    is_transpose=True,
                ).then_inc(sems["tp"], 1)
            # ---- compact quad B (ready during wdoc-lo)
            pe.wait_ge(sems["mul"], 2)
            for half in range(2):
                te.matmul(
                    ps_cpt[1].ap()[:, half * 512 : half * 512 + 512],
                    selv,
                    attn_q[1].ap()[:, half * 512 : half * 512 + 512],
                ).then_inc(sems["cpt"], 1)
            # ---- wdoc lo (b=0..3), 4-way column-tiled
            pe.wait_ge(sems["dNA"], 16)
            pe.wait_ge(sems["ttc"], 1)
            for t in range(ST):
                for j in range(4):
                    mm = te.matmul(
                        ps_wq[0].ap()[32 * j : 32 * j + A, :],
                        attnT.ap()[:, t, j * A : (j + 1) * A],
                        dN[0].ap()[:, j, t, :],
                        start=(t == 0),
                        stop=(t == ST - 1),
                        tile_position=(0, 32 * j),
                        skip_group_check=True,
                    )
            mm.then_inc(sems["wd"], 1)
            # ---- hi-half attn transposes
            pe.wait_ge(sems["bcp"], 4)
            for t in range(ST):
                te.matmul(
                    psT.ap()[:, t, :],
                    attn_ball.ap()[32:64, t * 128 : (t + 1) * 128],
                    idv[32:64, 32:64],
                    is_transpose=True,
                ).then_inc(sems["tp"], 1)
            # ---- wdoc hi
            pe.wait_ge(sems["dNB"], 16)
            pe.wait_ge(sems["ttc"], 2)
            for t in range(ST):
                for j in range(4):
                    mm = te.matmul(
                        ps_wq[1].ap()[32 * j : 32 * j + A, :],
                        attnT.ap()[:, t, (4 + j) * A : (5 + j) * A],
                        dN[1].ap()[:, j, t, :],
                        start=(t == 0),
                        stop=(t == ST - 1),
                        tile_position=(0, 32 * j),
                        skip_group_check=True,
                    )
            mm.then_inc(sems["wd"], 1)
            # ---- wall transposes, straight from the cast buffers
            for b in range(BL):
                pe.wait_ge(sems["wcp"], b + 1)
                te.matmul(
                    psWT.ap()[:, b * A : (b + 1) * A],
                    wtmp_all.ap()[:, b, :],
                    idv[0:A, 0:A],
                    is_transpose=True,
                ).then_inc(sems["wtp"], 1)
            # ---- rep
            pe.wait_ge(sems["wtc"], 1)
            pe.wait_ge(sems["apw"], 16)
            for a in range(A):
                if a >= 2:
                    prev = a - 2
                    if prev % 2 == 0:
                        pe.wait_ge(sems["repcv"], prev // 2 + 1)
                    else:
                        pe.wait_ge(sems["repcs"], prev // 2 + 1)
                te.matmul(
                    ps_rep[a % 2].ap()[:, :], wallT_v[:, :, a], apv[:, a, :]
                ).then_inc(sems["rep"], 1)

        @block.scalar
        def _(act):
            sc = nc.scalar
            act.wait_ge(sems["z"], 1)
            # dummy exp so the ACT table load happens before the scores land
            sc.activation(scr.ap()[:], zb.ap()[:], Exp, bias=zb.ap()[:, :])
            for q in range(2):
                act.wait_ge(sems["mm"], q + 1)
                sc.activation(
                    probs[q].ap()[:],
                    ps_q[q].ap()[:, :],
                    Exp,
                    bias=zb.ap()[:, :],
                    accum_out=sums_big.ap()[:, q : q + 1],
                ).then_inc(sems["exp"], 1)
                # 128-wide softmax scale (ACT is ~2.5x faster than DVE here)
                act.wait_ge(sems["rcp"], q + 1)
                sc.activation(
                    attn_q[q].ap()[:],
                    probs[q].ap()[:],
                    mybir.ActivationFunctionType.Copy,
                    scale=rs_big.ap()[:, q : q + 1],
                ).then_inc(sems["mul"], 1)

            act.wait_ge(sems["cpt"], 4)
            sc.copy(
                attn_ball.ap()[32:64, 512:1024], ps_cpt[1].ap()[:, 512:1024]
            ).then_inc(sems["bcp"], 1)
            # rep psum -> sbuf copies, odd aspects (even on DVE)
            for a in range(1, A, 2):
                act.wait_ge(sems["rep"], a + 1)
                sc.copy(rep_all.ap()[:, a, :], ps_rep[a % 2].ap()[:, :]).then_inc(
                    sems["repcs"], 1
                )

        @block.vector
        def _(dve):
            v = nc.vector
            v.memset(junk.ap()[:], 0.0).then_inc(sems["j"], 1)
            for q in range(2):
                dve.wait_ge(sems["exp"], q + 1)
                v.reciprocal(
                    rs_big.ap()[:, q : q + 1], sums_big.ap()[:, q : q + 1]
                ).then_inc(sems["rcp"], 1)
            # compacted rows -> attn_ball (legal bases 0/32)
            dve.wait_ge(sems["cpt"], 2)
            v.tensor_copy(attn_ball.ap()[0:32, :], ps_cpt[0].ap()[:, :]).then_inc(
                sems["bcp"], 2
            )
            # lo-half: copy the PE transposes out of PSUM
            dve.wait_ge(sems["tp"], ST)
            v.tensor_copy(attnT.ap()[:, :, 0:32], psT.ap()[:, :, :]).then_inc(
                sems["ttc"], 1
            )
            # (ii) quad-B attn copy before the wall staging copies
            dve.wait_ge(sems["cpt"], 4)
            v.tensor_copy(
                attn_ball.ap()[32:64, 0:512], ps_cpt[1].ap()[:, 0:512]
            ).then_inc(sems["bcp"], 1)
            dve.wait_ge(sems["wd"], 1)
            for j in range(4):
                v.tensor_copy(
                    wtmp_all.ap()[:, j, :], ps_wq[0].ap()[32 * j : 32 * j + A, :]
                ).then_inc(sems["wcp"], 1)
            dve.wait_ge(sems["tp"], 2 * ST)
            v.tensor_copy(attnT.ap()[:, :, 32:64], psT.ap()[:, :, :]).then_inc(
                sems["ttc"], 1
            )
            dve.wait_ge(sems["wd"], 2)
            for j in range(4):
                v.tensor_copy(
                    wtmp_all.ap()[:, 4 + j, :], ps_wq[1].ap()[32 * j : 32 * j + A, :]
                ).then_inc(sems["wcp"], 1)
            # wallT out of PSUM
            dve.wait_ge(sems["wtp"], BL)
            v.tensor_copy(wallT.ap()[:], psWT.ap()[:]).then_inc(sems["wtc"], 1)
            # rep psum -> sbuf copies, even aspects
            for a in range(0, A, 2):
                dve.wait_ge(sems["rep"], a + 1)
                v.tensor_copy(
                    rep_all.ap()[:, a, :], ps_rep[a % 2].ap()[:, :]
                ).then_inc(sems["repcv"], 1)

        @block.gpsimd
        def _(gp):
            nc.gpsimd.memset(zb.ap()[:], 0.0).then_inc(sems["z"], 1)

    nc.compile()
    return nc


def _get_nc():
    global _NC_CACHE
    if _NC_CACHE is None:
        _NC_CACHE = _build_nc()
    return _NC_CACHE


def make_in_maps(doc, aspE, aP):
    E = aspE.reshape(A, H, W)
    K = np.einsum("ahx,axw->ahw", aP, E)  # (A, H, W)
    kwid = np.zeros((128, 200), dtype=ml_dtypes.bfloat16)
    kwid[:, 0:128] = np.eye(128)
    kwid[:, 128 : 128 + W * A] = (
        K.transpose(1, 2, 0).reshape(H, W * A).astype(ml_dtypes.bfloat16)
    )
    for j in range(4):
        for a in range(A):
            kwid[32 * j + a, 168 + 8 * j + a] = 1
    apw = np.ascontiguousarray(aP.transpose(1, 0, 2).reshape(H, A * H)).astype(
        ml_dtypes.bfloat16
    )

    doc_bf = doc.astype(ml_dtypes.bfloat16)  # (B, S, H)
    in_maps = []
    for c in range(NCORES):
        dc = doc_bf[c * BL : (c + 1) * BL]  # (BL, S, H)
        m = {"kwid": kwid, "apw": apw}
        for i, half in enumerate(("A", "B")):
            dh = dc[i * 4 : (i + 1) * 4]
            # docN[p, b, t, h] = doc[b, t*128+p, h]
            m[f"docN{half}"] = np.ascontiguousarray(
                dh.reshape(4, ST, 128, H).transpose(2, 0, 1, 3)
            )
            dTc = np.zeros((4, 128, SP), dtype=ml_dtypes.bfloat16)
            dTc[:, :, PAD : PAD + S] = dh.transpose(0, 2, 1)
            m[f"docT{half}"] = dTc
        in_maps.append(m)
    return in_maps


def kernel(batch_docIn, aspEmbed_weight, aspProj):
    global LAST_RESULT
    doc = np.asarray(batch_docIn, dtype=np.float32)
    aspE = np.asarray(aspEmbed_weight, dtype=np.float32)
    aP = np.asarray(aspProj, dtype=np.float32)
    in_maps = make_in_maps(doc, aspE, aP)

    nc = _get_nc()
    res = run_bass_kernel_spmd(
        nc, in_maps, core_ids=list(range(NCORES)), trace=TRACE
    )
    LAST_RESULT = res

    attn = np.empty((B, A, S), dtype=np.float32)
    rep = np.empty((B, A, H), dtype=np.float32)
    for c in range(NCORES):
        attn[c * BL : (c + 1) * BL] = (
            res.results[c]["attn_out"].astype(np.float32).reshape(BL, A, S)
        )
        rep[c * BL : (c + 1) * BL] = res.results[c]["rep_out"]
    return attn, rep


# revision 34
# speedup vs baseline: 1.1343x; 1.0453x over previous
"""Trainium2 Bass kernel: per-aspect windowed attention (sparse_attention).

Reference math:
    proj[a,b,s,f] = sum_h doc[b,s,h] aspProj[a,h,f]
    score[a,b,s]  = sum_{w,f} proj[a,b,s+w-2,f] E[a,f,w]      (zero-padded in s)
    attn          = softmax_s(score)
    rep[a,b,f]    = sum_s proj[a,b,s,f] attn[a,b,s]

Algebraic refactor (proj never materialized):
    K[a,h,w]     = sum_f aspProj[a,h,f] E[a,f,w]              (tiny, host-side)
    score[a,b,s] = sum_{w,h} doc[b,s+w-2,h] K[a,h,w]
    attn         = exp(score) / rowsum                         (scores are tiny)
    rep[a,b,f]   = sum_h (sum_s attn[a,b,s] doc[b,s,h]) aspProj[a,h,f]

Sharding: data-parallel over batch, 8 batches per NeuronCore x 8 cores, no
collectives. Host pre-packs doc in bf16 twice: natural [s,h] tiles for the
s-contraction (wdoc) and transposed+padded [h,s] for the h-contraction
(scores). Raw bacc (no Tile framework) with hand-placed semaphores.

Score matmuls are 4-way column-tiled: batches j=0..3 of a quad run
concurrently in PE column groups (tile_position=(0,32j)), writing score
rows at partition 32j of one [128, S] psum pair. That packs M=8 output
rows x4 into the array and lets the softmax (exp+rowsum, reciprocal,
scale) run 128 partitions wide -- one op per quad instead of eight narrow
ones. Unused psum rows are zero-filled by the warmup matmuls so the
batched exp reads clean data.

attn is produced in bf16 (attn_out dram is bf16, upcast on host); scores
are tiny so exp(score) ~ 1 +- 0.2 and bf16 keeps ~3 decimal digits, well
inside the accuracy budget.
"""

import numpy as np
import ml_dtypes

import concourse.bass as bass
import concourse.bacc as bacc
import concourse.mybir as mybir
from concourse.bass_utils import run_bass_kernel_spmd

B, S, H, A, W = 64, 1024, 128, 8, 5
PAD = (W - 1) // 2
NCORES = 8
BL = B // NCORES          # local batches per core
SP = S + 2 * PAD          # padded seq width of docT
NPAIR = BL * A            # 64 (batch, aspect) pairs per core
ST = S // 128             # seq tiles per batch

BF16 = mybir.dt.bfloat16
FP32 = mybir.dt.float32

TRACE = False             # test.py flips this to profile
LAST_RESULT = None
N_WARMUP = 8              # N=512 warmup matmuls (~3.4us cold = HAM window)

_NC_CACHE = None


def _build_nc():
    nc = bacc.Bacc(
        "TRN2", target_bir_lowering=False, debug=False, num_devices=NCORES
    )
    Exp = mybir.ActivationFunctionType.Exp

    kwid_d = nc.dram_tensor("kwid", [128, 200], BF16, kind="ExternalInput")
    apw_d = nc.dram_tensor("apw", [128, 1024], BF16, kind="ExternalInput")
    dT_d = [
        nc.dram_tensor("docTA", [4, 128, SP], BF16, kind="ExternalInput"),
        nc.dram_tensor("docTB", [4, 128, SP], BF16, kind="ExternalInput"),
    ]
    dN_d = [
        nc.dram_tensor("docNA", [128, 4, ST, H], BF16, kind="ExternalInput"),
        nc.dram_tensor("docNB", [128, 4, ST, H], BF16, kind="ExternalInput"),
    ]
    attn_o = nc.dram_tensor("attn_out", [NPAIR, S], BF16, kind="ExternalOutput")
    rep_o = nc.dram_tensor("rep_out", [BL, A, H], FP32, kind="ExternalOutput")

    # ---- SBUF ----
    wt = nc.alloc_sbuf_tensor("wt_sb", [128, 1224], BF16)
    dT = [
        [nc.alloc_sbuf_tensor(f"dT{i}_{j}", [128, SP], BF16) for j in range(4)]
        for i in range(2)
    ]
    dN = [
        nc.alloc_sbuf_tensor(f"dN{i}", [128, 4, ST, H], BF16) for i in range(2)
    ]
    probs = [
        nc.alloc_sbuf_tensor(f"probs{q}", [128, S], FP32) for q in range(2)
    ]
    attn_q = [
        nc.alloc_sbuf_tensor(f"attnq{q}", [128, S], BF16) for q in range(2)
    ]
    attnT = nc.alloc_sbuf_tensor("attnT", [128, ST, NPAIR], BF16)
    sums_big = nc.alloc_sbuf_tensor("sums_big", [128, 2], FP32)
    rs_big = nc.alloc_sbuf_tensor("rs_big", [128, 2], FP32)
    zb = nc.alloc_sbuf_tensor("zb", [128, 1], FP32)
    scr = nc.alloc_sbuf_tensor("scr", [128, 1], FP32)
    wtmp_q = [
        nc.alloc_sbuf_tensor(f"wtmp{q}", [128, H], BF16) for q in range(2)
    ]
    wtmp_all = nc.alloc_sbuf_tensor("wtmp_all", [A, BL, H], BF16)
    junk = nc.alloc_sbuf_tensor("junk", [128, 512], BF16)
    wallT = nc.alloc_sbuf_tensor("wallT", [H, NPAIR], BF16)
    rep_all = nc.alloc_sbuf_tensor("rep_all", [BL, A, H], FP32)

    idv = wt.ap()[:, 0:128]
    kwv = wt.ap()[:, 128 : 128 + W * A].rearrange("h (w a) -> h w a", a=A)
    selv = wt.ap()[:, 168:200]
    apv = wt.ap()[:, 200 : 200 + A * H].rearrange("h (a f) -> h a f", f=H)
    wallT_v = wallT.ap().rearrange("h (b a) -> h b a", a=A)

    # ---- PSUM: quadA scores banks 0-1, quadB scores banks 2-3, warmup
    # bank 4 (also zero-fills the score banks), wdoc/rep rotation 5-7.
    # psT0/psT1/psWT alias the score banks (dead once exp has run).
    ps_q = [
        nc.place_psum_tensor(f"ps_q{q}", [128, S], FP32, bank=2 * q)
        for q in range(2)
    ]
    ps_warm = nc.place_psum_tensor("ps_warm", [128, 512], FP32, bank=4)
    ps_wq = [
        nc.place_psum_tensor(f"ps_wq{q}", [128, H], FP32, bank=5 + q)
        for q in range(2)
    ]
    ps_rep = [
        nc.place_psum_tensor("ps_rep0", [A, H], FP32, bank=7),
        nc.place_psum_tensor("ps_rep1", [A, H], FP32, bank=4),
    ]
    psT = nc.place_psum_tensor("psT", [128, ST, 32], FP32, bank=4)
    psWT = nc.place_psum_tensor("psWT", [128, NPAIR], BF16, bank=2)

    sems = {}
    sem_names = (
        ["kwid", "apw", "dNA", "dNB",
         "z", "j", "mm", "exp", "rcp", "mul", "tp", "ttc",
         "wd", "wcp", "wtp", "wtc", "rep", "repcv", "repcs",
         "oattnS", "oattnA", "orep"]
        + [f"dTA{j}" for j in range(4)]
        + [f"dTB{j}" for j in range(4)]
    )
    for name in sem_names:
        sems[name] = nc.alloc_semaphore(f"S_{name}")

    with nc.Block() as block:

        @block.sync
        def _(sync):
            sync.dma_start(wt.ap()[:, 0:200], kwid_d.ap()[:]).then_inc(
                sems["kwid"], 16
            )
            for i, nm in ((0, "dTA"), (1, "dTB")):
                for j in range(4):
                    sync.dma_start(
                        dT[i][j].ap()[:], dT_d[i].ap()[j]
                    ).then_inc(sems[f"{nm}{j}"], 16)
            sync.dma_start(dN[0].ap()[:], dN_d[0].ap()[:]).then_inc(
                sems["dNA"], 16
            )
            sync.dma_start(dN[1].ap()[:], dN_d[1].ap()[:]).then_inc(
                sems["dNB"], 16
            )
            sync.dma_start(wt.ap()[:, 200:1224], apw_d.ap()[:]).then_inc(
                sems["apw"], 16
            )
            # attn output DMAs straight from the scaled quads (j=2,3 on ACT)
            k = 0
            for q in range(2):
                sync.wait_ge(sems["mul"], q + 1)
                for j in (0, 1):
                    b = 4 * q + j
                    sync.wait_ge(sems["oattnS"], 16 * k)
                    sync.dma_start(
                        attn_o.ap()[b * A : (b + 1) * A, :],
                        attn_q[q].ap()[32 * j : 32 * j + A, :],
                    ).then_inc(sems["oattnS"], 16)
                    k += 1
            sync.wait_ge(sems["repcv"], 4)
            sync.wait_ge(sems["repcs"], 4)
            sync.dma_start(rep_o.ap()[:], rep_all.ap()[:]).then_inc(
                sems["orep"], 16
            )
            sync.wait_ge(sems["oattnS"], 64)
            sync.wait_ge(sems["oattnA"], 64)
            sync.wait_ge(sems["orep"], 16)

        @block.tensor
        def _(pe):
            te = nc.tensor
            # warmup on a zeroed junk tile: un-throttles the HAM while the
            # input DMAs fly, and zero-fills the psum banks that the
            # 128-wide consumers later read
            pe.wait_ge(sems["j"], 1)
            warm_tgts = [
                ps_q[0].ap()[:, 0:512],
                ps_q[0].ap()[:, 512:1024],
                ps_q[1].ap()[:, 0:512],
                ps_q[1].ap()[:, 512:1024],
                ps_wq[0].ap()[:, :],
                ps_wq[1].ap()[:, :],
            ]
            for i in range(N_WARMUP):
                tgt = warm_tgts[i] if i < 6 else ps_warm.ap()[:, :]
                te.matmul(
                    tgt,
                    junk.ap()[:, 0:128],
                    junk.ap()[:, 0 : tgt.shape[-1]],
                )
            # ---- scores: per quad, 4 batches run in parallel PE column
            # groups; 10 accumulating MMs per batch
            pe.wait_ge(sems["kwid"], 16)
            for q in range(2):
                for j in range(4):
                    pe.wait_ge(sems[("dTA" if q == 0 else "dTB") + str(j)], 16)
                for w in range(W):
                    for half in range(2):
                        for j in range(4):
                            mm = te.matmul(
                                ps_q[q].ap()[
                                    32 * j : 32 * j + A,
                                    half * 512 : half * 512 + 512,
                                ],
                                kwv[:, w, :],
                                dT[q][j].ap()[
                                    :, half * 512 + w : half * 512 + w + 512
                                ],
                                start=(w == 0),
                                stop=(w == W - 1),
                                tile_position=(0, 32 * j),
                                skip_group_check=True,
                            )
                mm.then_inc(sems["mm"], 1)
            # ---- quad-A attnT: compact+transpose in one matmul
            # out[s, pair] = sum_k attn_q[k, s] * sel[k, pair]
            pe.wait_ge(sems["mul"], 1)
            for t in range(ST):
                te.matmul(
                    psT.ap()[:, t, :],
                    attn_q[0].ap()[:, t * 128 : (t + 1) * 128],
                    selv,
                ).then_inc(sems["tp"], 1)
            # ---- wdoc lo (b=0..3), 4-way column-tiled
            pe.wait_ge(sems["dNA"], 16)
            pe.wait_ge(sems["ttc"], 1)
            for t in range(ST):
                for j in range(4):
                    mm = te.matmul(
                        ps_wq[0].ap()[32 * j : 32 * j + A, :],
                        attnT.ap()[:, t, j * A : (j + 1) * A],
                        dN[0].ap()[:, j, t, :],
                        start=(t == 0),
                        stop=(t == ST - 1),
                        tile_position=(0, 32 * j),
                        skip_group_check=True,
                    )
            mm.then_inc(sems["wd"], 1)
            # ---- quad-B attnT
            pe.wait_ge(sems["mul"], 2)
            for t in range(ST):
                te.matmul(
                    psT.ap()[:, t, :],
                    attn_q[1].ap()[:, t * 128 : (t + 1) * 128],
                    selv,
                ).then_inc(sems["tp"], 1)
            # ---- wdoc hi
            pe.wait_ge(sems["dNB"], 16)
            pe.wait_ge(sems["ttc"], 2)
            for t in range(ST):
                for j in range(4):
                    mm = te.matmul(
                        ps_wq[1].ap()[32 * j : 32 * j + A, :],
                        attnT.ap()[:, t, (4 + j) * A : (5 + j) * A],
                        dN[1].ap()[:, j, t, :],
                        start=(t == 0),
                        stop=(t == ST - 1),
                        tile_position=(0, 32 * j),
                        skip_group_check=True,
                    )
            mm.then_inc(sems["wd"], 1)
            # ---- wall transposes, straight from the cast buffers
            for b in range(BL):
                pe.wait_ge(sems["wcp"], b + 1)
                te.matmul(
                    psWT.ap()[:, b * A : (b + 1) * A],
                    wtmp_all.ap()[:, b, :],
                    idv[0:A, 0:A],
                    is_transpose=True,
                ).then_inc(sems["wtp"], 1)
            # ---- rep
            pe.wait_ge(sems["wtc"], 1)
            pe.wait_ge(sems["apw"], 16)
            for a in range(A):
                if a >= 2:
                    prev = a - 2
                    if prev % 2 == 0:
                        pe.wait_ge(sems["repcv"], prev // 2 + 1)
                    else:
                        pe.wait_ge(sems["repcs"], prev // 2 + 1)
                te.matmul(
                    ps_rep[a % 2].ap()[:, :], wallT_v[:, :, a], apv[:, a, :]
                ).then_inc(sems["rep"], 1)

        @block.scalar
        def _(act):
            sc = nc.scalar
            act.wait_ge(sems["z"], 1)
            # dummy exp so the ACT table load happens before the scores land
            sc.activation(scr.ap()[:], zb.ap()[:], Exp, bias=zb.ap()[:, :])
            for q in range(2):
                act.wait_ge(sems["mm"], q + 1)
                sc.activation(
                    probs[q].ap()[:],
                    ps_q[q].ap()[:, :],
                    Exp,
                    bias=zb.ap()[:, :],
                    accum_out=sums_big.ap()[:, q : q + 1],
                ).then_inc(sems["exp"], 1)
                # 128-wide softmax scale (ACT is ~2.5x faster than DVE here)
                act.wait_ge(sems["rcp"], q + 1)
                sc.activation(
                    attn_q[q].ap()[:],
                    probs[q].ap()[:],
                    mybir.ActivationFunctionType.Copy,
                    scale=rs_big.ap()[:, q : q + 1],
                ).then_inc(sems["mul"], 1)

            k = 0
            for q in range(2):
                act.wait_ge(sems["mul"], q + 1)
                for j in (2, 3):
                    b = 4 * q + j
                    act.wait_ge(sems["oattnA"], 16 * k)
                    act.dma_start(
                        attn_o.ap()[b * A : (b + 1) * A, :],
                        attn_q[q].ap()[32 * j : 32 * j + A, :],
                    ).then_inc(sems["oattnA"], 16)
                    k += 1
            # rep psum -> sbuf copies, odd aspects (even on DVE)
            for a in range(1, A, 2):
                act.wait_ge(sems["rep"], a + 1)
                sc.copy(rep_all.ap()[:, a, :], ps_rep[a % 2].ap()[:, :]).then_inc(
                    sems["repcs"], 1
                )

        @block.vector
        def _(dve):
            v = nc.vector
            v.memset(junk.ap()[:], 0.0).then_inc(sems["j"], 1)
            for q in range(2):
                dve.wait_ge(sems["exp"], q + 1)
                v.reciprocal(
                    rs_big.ap()[:, q : q + 1], sums_big.ap()[:, q : q + 1]
                ).then_inc(sems["rcp"], 1)
            # lo-half: copy the PE attnT matmuls out of PSUM
            dve.wait_ge(sems["tp"], ST)
            v.tensor_copy(attnT.ap()[:, :, 0:32], psT.ap()[:, :, :]).then_inc(
                sems["ttc"], 1
            )
            dve.wait_ge(sems["wd"], 1)
            for j in range(4):
                v.tensor_copy(
                    wtmp_all.ap()[:, j, :], ps_wq[0].ap()[32 * j : 32 * j + A, :]
                ).then_inc(sems["wcp"], 1)
            dve.wait_ge(sems["tp"], 2 * ST)
            v.tensor_copy(attnT.ap()[:, :, 32:64], psT.ap()[:, :, :]).then_inc(
                sems["ttc"], 1
            )
            dve.wait_ge(sems["wd"], 2)
            for j in range(4):
                v.tensor_copy(
                    wtmp_all.ap()[:, 4 + j, :], ps_wq[1].ap()[32 * j : 32 * j + A, :]
                ).then_inc(sems["wcp"], 1)
            # wallT out of PSUM
            dve.wait_ge(sems["wtp"], BL)
            v.tensor_copy(wallT.ap()[:], psWT.ap()[:]).then_inc(sems["wtc"], 1)
            # rep psum -> sbuf copies, even aspects
            for a in range(0, A, 2):
                dve.wait_ge(sems["rep"], a + 1)
                v.tensor_copy(
                    rep_all.ap()[:, a, :], ps_rep[a % 2].ap()[:, :]
                ).then_inc(sems["repcv"], 1)

        @block.gpsimd
        def _(gp):
            nc.gpsimd.memset(zb.ap()[:], 0.0).then_inc(sems["z"], 1)

    nc.compile()
    return nc


def _get_nc():
    global _NC_CACHE
    if _NC_CACHE is None:
        _NC_CACHE = _build_nc()
    return _NC_CACHE


def make_in_maps(doc, aspE, aP):
    E = aspE.reshape(A, H, W)
    K = np.einsum("ahx,axw->ahw", aP, E)  # (A, H, W)
    kwid = np.zeros((128, 200), dtype=ml_dtypes.bfloat16)
    kwid[:, 0:128] = np.eye(128)
    kwid[:, 128 : 128 + W * A] = (
        K.transpose(1, 2, 0).reshape(H, W * A).astype(ml_dtypes.bfloat16)
    )
    for j in range(4):
        for a in range(A):
            kwid[32 * j + a, 168 + 8 * j + a] = 1
    apw = np.ascontiguousarray(aP.transpose(1, 0, 2).reshape(H, A * H)).astype(
        ml_dtypes.bfloat16
    )

    doc_bf = doc.astype(ml_dtypes.bfloat16)  # (B, S, H)
    in_maps = []
    for c in range(NCORES):
        dc = doc_bf[c * BL : (c + 1) * BL]  # (BL, S, H)
        m = {"kwid": kwid, "apw": apw}
        for i, half in enumerate(("A", "B")):
            dh = dc[i * 4 : (i + 1) * 4]
            # docN[p, b, t, h] = doc[b, t*128+p, h]
            m[f"docN{half}"] = np.ascontiguousarray(
                dh.reshape(4, ST, 128, H).transpose(2, 0, 1, 3)
            )
            dTc = np.zeros((4, 128, SP), dtype=ml_dtypes.bfloat16)
            dTc[:, :, PAD : PAD + S] = dh.transpose(0, 2, 1)
            m[f"docT{half}"] = dTc
        in_maps.append(m)
    return in_maps


def kernel(batch_docIn, aspEmbed_weight, aspProj):
    global LAST_RESULT
    doc = np.asarray(batch_docIn, dtype=np.float32)
    aspE = np.asarray(aspEmbed_weight, dtype=np.float32)
    aP = np.asarray(aspProj, dtype=np.float32)
    in_maps = make_in_maps(doc, aspE, aP)

    nc = _get_nc()
    res = run_bass_kernel_spmd(
        nc, in_maps, core_ids=list(range(NCORES)), trace=TRACE
    )
    LAST_RESULT = res

    attn = np.empty((B, A, S), dtype=np.float32)
    rep = np.empty((B, A, H), dtype=np.float32)
    for c in range(NCORES):
        attn[c * BL : (c + 1) * BL] = (
            res.results[c]["attn_out"].astype(np.float32).reshape(BL, A, S)
        )
        rep[c * BL : (c + 1) * BL] = res.results[c]["rep_out"]
    return attn, rep
